# revision 28
# baseline (speedup 1.0000x reference)
"""Trainium2 Bass kernel for a binarized-conv BasicBlock (dense_cnn).

Computation (matches the reference nn.Module):
    out = clip(BN2(conv3x3(binarize(clip(BN1(conv3x3(binarize(x), binarize(w1))))),
                  binarize(w2)) + x))
with training-mode (batch-stats) BN over the full 64-image batch.

Strategy (v4):
  - Data-parallel over batch: 8 images per core on 8 NeuronCores.
  - Binarized 3x3 conv as 9 accumulating DoubleRow fp8 PE matmuls (K=256)
    per [128, 392] output tile over zero-padded [128, 2, 30, 32] fp8
    activations; +-0.5/1 values in fp8 are exact, accumulation in fp32 PSUM.
  - BN1 + hardtanh + binarize collapses to a per-channel threshold compare.
  - conv1 runs pair-major (x streams from HBM across both HWDGE queues;
    each image pair lands ~8us before its groups need it).  Sync-BN1 is one
    AllReduce at conv1 end, its wait filled with the w2 weight prep; the
    conv2-input binarize is paced pair-wise into the first conv2 groups.
  - conv2 runs OB-MAJOR: AllReduce(ob0 stats) + the final affine/clamp/
    store of the ob0 channels hide under conv2's ob1 matmuls; the tail is
    only AllReduce(ob1) + the ob1 affine+store pipeline.  The ob0 eviction
    squares run on gpsimd so the ob0 stats close immediately.
  - Warmup collective is dependency-free (fires at ~8us, ncfw wake done
    before the first real AllReduce).  Stat uploads + result fetches ride
    the sync HWDGE queue (fast completion path); emission order keeps every
    queue's waits monotone so nothing stalls behind them.
"""

import os
import sys

import numpy as np


def _ensure_paths():
    for p in ("/opt/trn_rl_repo", "/root/.axon_site/_ro/trn_rl_repo"):
        if p not in sys.path and os.path.isdir(p):
            sys.path.append(p)


try:
    from concourse import bacc, mybir, tile  # noqa: F401
except ImportError:
    _ensure_paths()
    from concourse import bacc, mybir, tile  # noqa: F401

from concourse.bass_utils import run_bass_kernel_spmd
from concourse.masks import make_identity

N_CORES = 8
IMGS = 8          # images per core (64 / 8)
C = 256
CB = 2            # channel blocks of 128
H = W = 28
HP = WP = 30      # zero-padded spatial
PIX = H * W       # 784
HALF = PIX // 2   # 392 (one PSUM bank of fp32)
NT = 64 * PIX     # BN count over the GLOBAL batch (N*H*W)
EPS = 1e-5

F32 = mybir.dt.float32
FP8 = mybir.dt.float8e4
AF = mybir.ActivationFunctionType
ALU = mybir.AluOpType
DR = mybir.MatmulPerfMode.DoubleRow

# padded fp8 activation layout: [128, 2 kblocks, 30 rows, 32 cols]
RP = 32           # row pitch (28 cols + pad, %16 bytes)
KP = HP * RP      # per-kblock pitch = 960

_PROGRAM = None


def _build_program():
    nc = bacc.Bacc("TRN2", target_bir_lowering=False, debug=False,
                   num_devices=N_CORES)

    x_in = nc.dram_tensor("x", [IMGS, C, H, W], F32, kind="ExternalInput").ap()
    w1_in = nc.dram_tensor("w1", [C, C, 3, 3], F32, kind="ExternalInput").ap()
    w2_in = nc.dram_tensor("w2", [C, C, 3, 3], F32, kind="ExternalInput").ap()
    g1_in = nc.dram_tensor("gamma1", [C], F32, kind="ExternalInput").ap()
    b1_in = nc.dram_tensor("beta1", [C], F32, kind="ExternalInput").ap()
    g2_in = nc.dram_tensor("gamma2", [C], F32, kind="ExternalInput").ap()
    b2_in = nc.dram_tensor("beta2", [C], F32, kind="ExternalInput").ap()
    out_d = nc.dram_tensor("out", [IMGS, C, H, W], F32, kind="ExternalOutput").ap()

    groups = [list(range(N_CORES))]

    with tile.TileContext(nc) as tc:
        with (
            tc.tile_pool(name="consts", bufs=1) as p_const,
            tc.tile_pool(name="wstage", bufs=2) as p_wstage,
            tc.tile_pool(name="wt", bufs=2 * 9 * 2) as p_wt,
            tc.tile_pool(name="xp", bufs=IMGS * CB) as p_x,
            tc.tile_pool(name="apad", bufs=2 * IMGS) as p_apad,
            tc.tile_pool(name="yz", bufs=IMGS * CB) as p_yz,
            tc.tile_pool(name="sq", bufs=3) as p_sq,
            tc.tile_pool(name="o1", bufs=4) as p_o1,
            tc.tile_pool(name="ps", bufs=8, space="PSUM") as p_ps,
            tc.tile_pool(name="dram", bufs=1, space="DRAM") as p_dram,
        ):
            # ---- warmup collective: dependency-free (reads an uninitialized
            # DRAM scratch tile, result unused) so gpsimd triggers it
            # immediately and the ncfw wake is done before the first real
            # AllReduce.
            ccw_i = p_dram.tile([128, 1], F32, name="ccw_i")
            ccw_o = p_dram.tile([128, 1], F32, name="ccw_o")
            nc.gpsimd.collective_compute(
                "AllReduce", ALU.add, replica_groups=groups,
                ins=[ccw_i.opt()], outs=[ccw_o.opt()])

            ident = p_const.tile([128, 128], F32, name="ident")
            make_identity(nc, ident)

            # kb-aligned staging chunks so the per-kb transposes can start
            # before the other kb arrives.
            def stage_w(w_in, ob, nm, eng, parts=4):
                wst = p_wstage.tile([128, C * 9], F32, tag="wst", name=nm)
                n = C * 9
                step = n // parts
                for a in range(0, n, step):
                    eng.dma_start(
                        out=wst[:, a:a + step],
                        in_=w_in[ob * 128:(ob + 1) * 128].rearrange(
                            "o i ky kx -> o (i ky kx)")[:, a:a + step])
                return wst

            wst1_0 = stage_w(w1_in, 0, "wst1_0", nc.sync)
            wst1_1 = stage_w(w1_in, 1, "wst1_1", nc.scalar)

            epsc = p_const.tile([128, 1], F32, name="epsc")
            nc.vector.memset(epsc, float(EPS))

            # ---- padded fp8 buffers; memsets all on gpsimd, image order ----
            xsign = [None] * IMGS
            b2a = [None] * IMGS
            for n in range(IMGS):
                ap = p_apad.tile([128, CB * KP], FP8, tag="apad",
                                 name=f"xs_{n}")
                nc.gpsimd.memset(ap, 0.0)
                xsign[n] = ap
            for n in range(IMGS):
                ap = p_apad.tile([128, CB * KP], FP8, tag="apad",
                                 name=f"b2_{n}")
                nc.gpsimd.memset(ap, 0.0)
                b2a[n] = ap

            # per-channel stat accumulators, one column per (img, half)
            def stat_tiles(nm):
                return [p_const.tile([128, IMGS * 2], F32, name=f"{nm}{ob}")
                        for ob in range(CB)]

            st1s, st1q = stat_tiles("st1s"), stat_tiles("st1q")
            st2s, st2q = stat_tiles("st2s"), stat_tiles("st2q")

            # ---- weight prep: sign(w)^T as DoubleRow fp8 [128 i, 2 kb, 128 o]
            def prep_weights(wst, wi, ob, wt):
                w3 = wst.rearrange("p (i t) -> p i t", t=9)
                for tap in range(9):
                    t = p_wt.tile([128, CB * 128], FP8, tag="wt",
                                  name=f"wt{wi}_{tap}_{ob}")
                    wt[(tap, ob)] = t
                    for kb in range(CB):
                        ps = p_ps.tile([128, 128], F32, tag="ps",
                                       name=f"pst{wi}_{ob}_{kb}_{tap}")
                        nc.tensor.transpose(
                            ps, w3[:, kb * 128:(kb + 1) * 128, tap], ident)
                        nc.scalar.activation(
                            t[:, kb * 128:(kb + 1) * 128], ps, AF.Sign)
                return wt

            # ---- binarize x into padded fp8: per (img, block) DVE op ----
            def binz_x(n):
                a4 = xsign[n].rearrange("p (k r c) -> p k r c", k=CB, r=HP)
                for b in range(CB):
                    nc.vector.tensor_scalar(
                        out=a4[:, b, 1:29, 1:29],
                        in0=xt[n][b].rearrange("p (h w) -> p h w", h=H),
                        scalar1=0.0, scalar2=0.5,
                        op0=ALU.is_ge, op1=ALU.subtract)

            # ---- conv: 9 DoubleRow matmuls (K=256) per [128, 392] PSUM tile
            def emit_group(wt, act, evict, pair, ob):
                tiles = [(n, half)
                         for n in (2 * pair, 2 * pair + 1)
                         for half in range(2)]
                pss = {}
                for (n, half) in tiles:
                    pss[(n, half)] = p_ps.tile(
                        [128, HALF], F32, tag="ps",
                        name=f"ps_{ob}_{n}_{half}")
                for tap in range(9):
                    dy, dx = divmod(tap, 3)
                    w3 = wt[(tap, ob)].rearrange(
                        "p (k o) -> p k o", k=CB)
                    for (n, half) in tiles:
                        a4 = act[n].rearrange(
                            "p (k r c) -> p k r c", k=CB, r=HP)
                        rhs = a4[:, :, dy + half * 14: dy + half * 14 + 14,
                                 dx: dx + W]
                        nc.tensor.matmul(pss[(n, half)], w3, rhs,
                                         start=(tap == 0),
                                         stop=(tap == 8),
                                         perf_mode=DR)
                for (n, half) in tiles:
                    evict(n, ob, half, pss[(n, half)])

            # ---- conv1 eviction: copy PSUM->y1 with sum, square with sumsq
            y1 = [[None] * CB for _ in range(IMGS)]

            def evict1(n, ob, half, ps):
                if y1[n][ob] is None:
                    y1[n][ob] = p_yz.tile([128, PIX], F32, tag="yz",
                                          name=f"y1_{n}_{ob}")
                idx = n * 2 + half
                ysl = y1[n][ob][:, half * HALF:(half + 1) * HALF]
                nc.scalar.activation(ysl, ps, AF.Copy, scale=2.0,
                                     accum_out=st1s[ob][:, idx:idx + 1])
                sq = p_sq.tile([128, HALF], F32, tag="sq")
                nc.vector.scalar_tensor_tensor(
                    out=sq, in0=ysl, scalar=1.0, in1=ysl,
                    op0=ALU.mult, op1=ALU.mult,
                    accum_out=st1q[ob][:, idx:idx + 1])

            # ---- stat reduce + AllReduce start for the given ob list.
            # cci rides the sync HWDGE queue: its completion semaphore fires
            # ~1us after issue (the SWDGE path takes ~8us), so the gpsimd
            # collective triggers almost immediately.
            def sync_stats(ss, qq, obs, nm):
                w = 2 * len(obs)
                pk = p_const.tile([128, w], F32, name=f"pk{nm}")
                for j, ob in enumerate(obs):
                    nc.vector.tensor_reduce(out=pk[:, 2 * j:2 * j + 1],
                                            in_=ss[ob],
                                            axis=mybir.AxisListType.X,
                                            op=ALU.add)
                    nc.vector.tensor_reduce(out=pk[:, 2 * j + 1:2 * j + 2],
                                            in_=qq[ob],
                                            axis=mybir.AxisListType.X,
                                            op=ALU.add)
                cci = p_dram.tile([128, w], F32, name=f"cci{nm}")
                cco = p_dram.tile([128, w], F32, name=f"cco{nm}")
                nc.sync.dma_start(out=cci, in_=pk)
                # poke: a tiny gpsimd op dependent on pk lands the gpsimd
                # sequencer right at stats-close, so the collective's
                # semaphore wait starts (and is observed) promptly
                poke = p_const.tile([128, 1], F32, name=f"poke{nm}")
                nc.gpsimd.tensor_scalar(out=poke, in0=pk[:, 0:1], scalar1=1.0,
                                        scalar2=None, op0=ALU.mult)
                nc.gpsimd.collective_compute(
                    "AllReduce", ALU.add, replica_groups=groups,
                    ins=[cci.opt()], outs=[cco.opt()])
                return cco

            def fetch_stats(cco, nm, w=2):
                red = p_const.tile([128, w], F32, name=f"red{nm}")
                nc.sync.dma_start(out=red, in_=cco)
                return red

            # ---- BN1 threshold from global sums (cols 2j:2j+2 of red) ----
            def thr_chain(red, j, ob, nm):
                m = p_const.tile([128, 1], F32, name=f"m{nm}")
                nc.vector.tensor_scalar(out=m, in0=red[:, 2 * j:2 * j + 1],
                                        scalar1=1.0 / NT, scalar2=None,
                                        op0=ALU.mult)
                mm = p_const.tile([128, 1], F32, name=f"mm{nm}")
                nc.vector.tensor_mul(mm, m, m)
                v = p_const.tile([128, 1], F32, name=f"v{nm}")
                nc.vector.scalar_tensor_tensor(
                    out=v, in0=red[:, 2 * j + 1:2 * j + 2], scalar=1.0 / NT,
                    in1=mm, op0=ALU.mult, op1=ALU.subtract)
                sd = p_const.tile([128, 1], F32, name=f"sd{nm}")
                nc.scalar.activation(sd, v, AF.Sqrt, bias=epsc)
                tb = p_const.tile([128, 1], F32, name=f"tb{nm}")
                nc.vector.tensor_mul(tb, bg1[:, ob:ob + 1], sd)
                thr = p_const.tile([128, 1], F32, name=f"thr{nm}")
                nc.vector.tensor_sub(thr, m, tb)
                return thr

            # binarize(BN1(y1[., ob])) == is_ge(y1, thr) - 0.5 into kb plane.
            # Runs on gpsimd (idle during conv2) so the DVE eviction pipeline
            # never backs up.
            def binz_y(n, ob, thr):
                a4 = b2a[n].rearrange("p (k r c) -> p k r c", k=CB, r=HP)
                nc.gpsimd.tensor_scalar(
                    out=a4[:, ob, 1:29, 1:29],
                    in0=y1[n][ob].rearrange("p (h w) -> p h w", h=H),
                    scalar1=thr, scalar2=0.5,
                    op0=ALU.is_ge, op1=ALU.subtract)

            # ================= conv1 (pair-major) =================
            wt1 = {}
            # signs of w1-ob0 go FIRST in the scalar queue, before any
            # throttled bulk-DMA issue instructions
            prep_weights(wst1_0, 1, 0, wt1)

            # ---- x: one DMA per (image, block), alternating the two HWDGE
            # queues; image n lands well before its pair's groups.
            # all x on the sync queue: its throttled issues block nothing,
            # and the scalar queue stays clear for the weight signs
            xt = [[None] * CB for _ in range(IMGS)]   # [n][b]
            for n in range(IMGS):
                for b in range(CB):
                    xr = p_x.tile([128, PIX], F32, tag="xp",
                                  name=f"x_{n}_{b}")
                    nc.sync.dma_start(
                        out=xr,
                        in_=x_in[n, b * 128:(b + 1) * 128].rearrange(
                            "c h w -> c (h w)"))
                    xt[n][b] = xr

            # w2 on sync behind the even x halves (needed only at conv1 end)
            wst2_0 = stage_w(w2_in, 0, "wst2_0", nc.sync)
            wst2_1 = stage_w(w2_in, 1, "wst2_1", nc.sync)

            # gamma/beta as [128, 2] (col = channel block), scalar queue
            def load_cvec(src, nm):
                t = p_const.tile([128, CB], F32, name=nm)
                nc.scalar.dma_start(out=t,
                                    in_=src.rearrange("(b p) -> p b", p=128))
                return t

            g1t = load_cvec(g1_in, "g1t")
            b1t = load_cvec(b1_in, "b1t")
            g2t = load_cvec(g2_in, "g2t")
            b2t = load_cvec(b2_in, "b2t")

            binz_x(0)
            binz_x(1)
            emit_group(wt1, xsign, evict1, 0, 0)
            prep_weights(wst1_1, 1, 1, wt1)
            for pair in range(4):
                if pair > 0:
                    emit_group(wt1, xsign, evict1, pair, 0)
                # next pair's binarize sits between this pair's ob0/ob1
                # eviction squares in the DVE queue: runs as soon as its x
                # slices land, never stalling the eviction pipeline
                if pair < 3:
                    binz_x(2 * pair + 2)
                    binz_x(2 * pair + 3)
                emit_group(wt1, xsign, evict1, pair, 1)

            # BN1: one AllReduce for both channel blocks
            cco1 = sync_stats(st1s, st1q, [0, 1], "1")
            # w2 prep fills the AllReduce wait on PE/Scalar
            wt2 = {}
            prep_weights(wst2_0, 2, 0, wt2)
            prep_weights(wst2_1, 2, 1, wt2)
            red1 = fetch_stats(cco1, "1", w=4)

            rg1 = p_const.tile([128, CB], F32, name="rg1")
            nc.vector.reciprocal(rg1, g1t)
            bg1 = p_const.tile([128, CB], F32, name="bg1")
            nc.vector.tensor_mul(bg1, b1t, rg1)
            thr1 = [thr_chain(red1, j, j, f"1_{j}") for j in range(CB)]

            # ---- conv2 eviction: z = 2*psum + x (fused sum), square.
            # ob0 squares on gpsimd (idle then) so the ob0 stats close
            # immediately after the last eviction; ob1 squares on scalar.
            z = [[None] * CB for _ in range(IMGS)]

            def evict2(n, ob, half, ps):
                if z[n][ob] is None:
                    z[n][ob] = p_yz.tile([128, PIX], F32, tag="yz",
                                         name=f"z_{n}_{ob}")
                idx = n * 2 + half
                zsl = z[n][ob][:, half * HALF:(half + 1) * HALF]
                nc.vector.scalar_tensor_tensor(
                    out=zsl, in0=ps, scalar=2.0,
                    in1=xt[n][ob][:, half * HALF:(half + 1) * HALF],
                    op0=ALU.mult, op1=ALU.add,
                    accum_out=st2s[ob][:, idx:idx + 1])
                sq = p_sq.tile([128, HALF], F32, tag="sq")
                nc.scalar.activation(sq, zsl, AF.Square,
                                     accum_out=st2q[ob][:, idx:idx + 1])

            # ---- final: out = clip(z * fscale + fbias), one DMA per (n,ob)
            def affine_chain(red, ob, nm):
                m = p_const.tile([128, 1], F32, name=f"m{nm}")
                nc.vector.tensor_scalar(out=m, in0=red[:, 0:1],
                                        scalar1=1.0 / NT, scalar2=None,
                                        op0=ALU.mult)
                mm = p_const.tile([128, 1], F32, name=f"mm{nm}")
                nc.vector.tensor_mul(mm, m, m)
                v = p_const.tile([128, 1], F32, name=f"v{nm}")
                nc.vector.scalar_tensor_tensor(
                    out=v, in0=red[:, 1:2], scalar=1.0 / NT, in1=mm,
                    op0=ALU.mult, op1=ALU.subtract)
                sd = p_const.tile([128, 1], F32, name=f"sd{nm}")
                nc.scalar.activation(sd, v, AF.Sqrt, bias=epsc)
                rstd = p_const.tile([128, 1], F32, name=f"rstd{nm}")
                nc.vector.reciprocal(rstd, sd)
                fsc = p_const.tile([128, 1], F32, name=f"fsc{nm}")
                nc.vector.tensor_mul(fsc, g2t[:, ob:ob + 1], rstd)
                msc = p_const.tile([128, 1], F32, name=f"msc{nm}")
                nc.vector.tensor_mul(msc, m, fsc)
                fb = p_const.tile([128, 1], F32, name=f"fb{nm}")
                nc.vector.tensor_sub(fb, b2t[:, ob:ob + 1], msc)
                return fsc, fb

            def store_out(n, ob, fsc, fb, eng):
                o1 = p_o1.tile([128, PIX], F32, tag="o1")
                nc.scalar.activation(o1, z[n][ob], AF.Identity,
                                     bias=fb, scale=fsc)
                nc.vector.tensor_scalar(out=o1, in0=o1, scalar1=-1.0,
                                        scalar2=1.0, op0=ALU.max,
                                        op1=ALU.min)
                eng.dma_start(
                    out=out_d[n, ob * 128:(ob + 1) * 128].rearrange(
                        "c h w -> c (h w)"),
                    in_=o1)

            # ================= conv2 (ob-major) =================
            # binarize paced with the first ob0 groups
            for pair in range(4):
                for n in (2 * pair, 2 * pair + 1):
                    binz_y(n, 0, thr1[0])
                    binz_y(n, 1, thr1[1])
                emit_group(wt2, b2a, evict2, pair, 0)

            cco2a = sync_stats(st2s, st2q, [0], "2a")

            emit_group(wt2, b2a, evict2, 0, 1)
            emit_group(wt2, b2a, evict2, 1, 1)
            emit_group(wt2, b2a, evict2, 2, 1)
            # AllReduce(2a) done by now: ob0 affine+clamp+store under conv2-ob1
            red2a = fetch_stats(cco2a, "2a")
            fsc0, fb0 = affine_chain(red2a, 0, "2a")
            for n in range(IMGS):
                store_out(n, 0, fsc0, fb0, nc.sync if n % 2 == 0 else nc.gpsimd)
            emit_group(wt2, b2a, evict2, 3, 1)

            cco2b = sync_stats(st2s, st2q, [1], "2b")
            red2b = fetch_stats(cco2b, "2b")
            fsc1, fb1 = affine_chain(red2b, 1, "2b")
            for n in range(IMGS):
                store_out(n, 1, fsc1, fb1, nc.sync if n % 2 == 0 else nc.gpsimd)

    nc.compile()
    return nc


def _get_program():
    global _PROGRAM
    if _PROGRAM is None:
        _PROGRAM = _build_program()
    return _PROGRAM


def run_sharded(inputs, **spmd_kwargs):
    """Shard inputs across 8 cores, run, and gather. Returns (out, results)."""
    nc = _get_program()
    x = np.ascontiguousarray(np.asarray(inputs["x"], dtype=np.float32))
    base = {
        k: np.ascontiguousarray(np.asarray(inputs[k], dtype=np.float32))
        for k in ("w1", "w2", "gamma1", "beta1", "gamma2", "beta2")
    }
    shards = np.split(x, N_CORES, axis=0)
    in_maps = [{"x": shards[i], **base} for i in range(N_CORES)]
    res = run_bass_kernel_spmd(nc, in_maps, core_ids=list(range(N_CORES)),
                               **spmd_kwargs)
    out = np.concatenate([res.results[i]["out"] for i in range(N_CORES)],
                         axis=0).astype(np.float32)
    return out, res


def kernel(**inputs):
    out, _ = run_sharded(inputs)
    return out


# revision 29
# speedup vs baseline: 1.6975x; 1.6975x over previous
"""Trainium2 Bass kernel for a binarized-conv BasicBlock (dense_cnn).

Computation (matches the reference nn.Module):
    out = clip(BN2(conv3x3(binarize(clip(BN1(conv3x3(binarize(x), binarize(w1))))),
                  binarize(w2)) + x))
with training-mode (batch-stats) BN over the full 64-image batch.

Strategy (v4):
  - Data-parallel over batch: 8 images per core on 8 NeuronCores.
  - Binarized 3x3 conv as 9 accumulating DoubleRow fp8 PE matmuls (K=256)
    per [128, 392] output tile over zero-padded [128, 2, 30, 32] fp8
    activations; +-0.5/1 values in fp8 are exact, accumulation in fp32 PSUM.
  - BN1 + hardtanh + binarize collapses to a per-channel threshold compare.
  - conv1 runs pair-major (x streams from HBM across both HWDGE queues;
    each image pair lands ~8us before its groups need it).  Sync-BN1 is one
    AllReduce at conv1 end, its wait filled with the w2 weight prep; the
    conv2-input binarize is paced pair-wise into the first conv2 groups.
  - conv2 runs OB-MAJOR: AllReduce(ob0 stats) + the final affine/clamp/
    store of the ob0 channels hide under conv2's ob1 matmuls; the tail is
    only AllReduce(ob1) + the ob1 affine+store pipeline.  The ob0 eviction
    squares run on gpsimd so the ob0 stats close immediately.
  - Warmup collective is dependency-free (fires at ~8us, ncfw wake done
    before the first real AllReduce).  Stat uploads + result fetches ride
    the sync HWDGE queue (fast completion path); emission order keeps every
    queue's waits monotone so nothing stalls behind them.
"""

import os
import sys

import numpy as np


def _ensure_paths():
    for p in ("/opt/trn_rl_repo", "/root/.axon_site/_ro/trn_rl_repo"):
        if p not in sys.path and os.path.isdir(p):
            sys.path.append(p)


try:
    from concourse import bacc, mybir, tile  # noqa: F401
except ImportError:
    _ensure_paths()
    from concourse import bacc, mybir, tile  # noqa: F401

from concourse.bass_utils import run_bass_kernel_spmd
from concourse.masks import make_identity

N_CORES = 8
IMGS = 8          # images per core (64 / 8)
C = 256
CB = 2            # channel blocks of 128
H = W = 28
HP = WP = 30      # zero-padded spatial
PIX = H * W       # 784
HALF = PIX // 2   # 392 (one PSUM bank of fp32)
NT = 64 * PIX     # BN count over the GLOBAL batch (N*H*W)
EPS = 1e-5

F32 = mybir.dt.float32
FP8 = mybir.dt.float8e4
AF = mybir.ActivationFunctionType
ALU = mybir.AluOpType
DR = mybir.MatmulPerfMode.DoubleRow

# padded fp8 activation layout: [128, 2 kblocks, 30 rows, 32 cols]
RP = 32           # row pitch (28 cols + pad, %16 bytes)
KP = HP * RP      # per-kblock pitch = 960

_PROGRAM = None


def _build_program():
    nc = bacc.Bacc("TRN2", target_bir_lowering=False, debug=False,
                   num_devices=N_CORES)

    x_in = nc.dram_tensor("x", [IMGS, C, H, W], F32, kind="ExternalInput").ap()
    w1_in = nc.dram_tensor("w1", [C, C, 3, 3], F32, kind="ExternalInput").ap()
    w2_in = nc.dram_tensor("w2", [C, C, 3, 3], F32, kind="ExternalInput").ap()
    g1_in = nc.dram_tensor("gamma1", [C], F32, kind="ExternalInput").ap()
    b1_in = nc.dram_tensor("beta1", [C], F32, kind="ExternalInput").ap()
    g2_in = nc.dram_tensor("gamma2", [C], F32, kind="ExternalInput").ap()
    b2_in = nc.dram_tensor("beta2", [C], F32, kind="ExternalInput").ap()
    out_d = nc.dram_tensor("out", [IMGS, C, H, W], F32, kind="ExternalOutput").ap()

    groups = [list(range(N_CORES))]

    with tile.TileContext(nc) as tc:
        with (
            tc.tile_pool(name="consts", bufs=1) as p_const,
            tc.tile_pool(name="wstage", bufs=2) as p_wstage,
            tc.tile_pool(name="wt", bufs=2 * 9 * 2) as p_wt,
            tc.tile_pool(name="xp", bufs=IMGS * CB) as p_x,
            tc.tile_pool(name="apad", bufs=2 * IMGS) as p_apad,
            tc.tile_pool(name="yz", bufs=IMGS * CB) as p_yz,
            tc.tile_pool(name="sq", bufs=3) as p_sq,
            tc.tile_pool(name="o1", bufs=4) as p_o1,
            tc.tile_pool(name="ps", bufs=8, space="PSUM") as p_ps,
            tc.tile_pool(name="dram", bufs=1, space="DRAM") as p_dram,
        ):
            # ---- warmup collective: dependency-free (reads an uninitialized
            # DRAM scratch tile, result unused) so gpsimd triggers it
            # immediately and the ncfw wake is done before the first real
            # AllReduce.
            ccw_i = p_dram.tile([128, 1], F32, name="ccw_i")
            ccw_o = p_dram.tile([128, 1], F32, name="ccw_o")
            nc.gpsimd.collective_compute(
                "AllReduce", ALU.add, replica_groups=groups,
                ins=[ccw_i.opt()], outs=[ccw_o.opt()])

            ident = p_const.tile([128, 128], F32, name="ident")
            make_identity(nc, ident)

            # kb-aligned staging chunks so the per-kb transposes can start
            # before the other kb arrives.
            def stage_w(w_in, ob, nm, eng, parts=4):
                wst = p_wstage.tile([128, C * 9], F32, tag="wst", name=nm)
                n = C * 9
                step = n // parts
                for a in range(0, n, step):
                    eng.dma_start(
                        out=wst[:, a:a + step],
                        in_=w_in[ob * 128:(ob + 1) * 128].rearrange(
                            "o i ky kx -> o (i ky kx)")[:, a:a + step])
                return wst

            wst1_0 = stage_w(w1_in, 0, "wst1_0", nc.sync)
            wst1_1 = stage_w(w1_in, 1, "wst1_1", nc.scalar)

            epsc = p_const.tile([128, 1], F32, name="epsc")
            nc.vector.memset(epsc, float(EPS))

            # ---- padded fp8 buffers; memsets all on gpsimd, image order ----
            xsign = [None] * IMGS
            b2a = [None] * IMGS
            for n in range(IMGS):
                ap = p_apad.tile([128, CB * KP], FP8, tag="apad",
                                 name=f"xs_{n}")
                nc.gpsimd.memset(ap, 0.0)
                xsign[n] = ap
            for n in range(IMGS):
                ap = p_apad.tile([128, CB * KP], FP8, tag="apad",
                                 name=f"b2_{n}")
                nc.gpsimd.memset(ap, 0.0)
                b2a[n] = ap

            # per-channel stat accumulators, one column per (img, half)
            def stat_tiles(nm):
                return [p_const.tile([128, IMGS * 2], F32, name=f"{nm}{ob}")
                        for ob in range(CB)]

            st1s, st1q = stat_tiles("st1s"), stat_tiles("st1q")
            st2s, st2q = stat_tiles("st2s"), stat_tiles("st2q")

            # ---- weight prep: sign(w)^T as DoubleRow fp8 [128 i, 2 kb, 128 o]
            def prep_weights(wst, wi, ob, wt):
                w3 = wst.rearrange("p (i t) -> p i t", t=9)
                for tap in range(9):
                    t = p_wt.tile([128, CB * 128], FP8, tag="wt",
                                  name=f"wt{wi}_{tap}_{ob}")
                    wt[(tap, ob)] = t
                    for kb in range(CB):
                        ps = p_ps.tile([128, 128], F32, tag="ps",
                                       name=f"pst{wi}_{ob}_{kb}_{tap}")
                        nc.tensor.transpose(
                            ps, w3[:, kb * 128:(kb + 1) * 128, tap], ident)
                        nc.scalar.activation(
                            t[:, kb * 128:(kb + 1) * 128], ps, AF.Sign)
                return wt

            # ---- binarize x into padded fp8: per (img, block) DVE op ----
            def binz_x(n):
                a4 = xsign[n].rearrange("p (k r c) -> p k r c", k=CB, r=HP)
                for b in range(CB):
                    nc.vector.tensor_scalar(
                        out=a4[:, b, 1:29, 1:29],
                        in0=xt[n][b].rearrange("p (h w) -> p h w", h=H),
                        scalar1=0.0, scalar2=0.5,
                        op0=ALU.is_ge, op1=ALU.subtract)

            # ---- conv: 9 DoubleRow matmuls (K=256) per [128, 392] PSUM tile
            def emit_group(wt, act, evict, pair, ob):
                tiles = [(n, half)
                         for n in (2 * pair, 2 * pair + 1)
                         for half in range(2)]
                pss = {}
                for (n, half) in tiles:
                    pss[(n, half)] = p_ps.tile(
                        [128, HALF], F32, tag="ps",
                        name=f"ps_{ob}_{n}_{half}")
                for tap in range(9):
                    dy, dx = divmod(tap, 3)
                    w3 = wt[(tap, ob)].rearrange(
                        "p (k o) -> p k o", k=CB)
                    for (n, half) in tiles:
                        a4 = act[n].rearrange(
                            "p (k r c) -> p k r c", k=CB, r=HP)
                        rhs = a4[:, :, dy + half * 14: dy + half * 14 + 14,
                                 dx: dx + W]
                        nc.tensor.matmul(pss[(n, half)], w3, rhs,
                                         start=(tap == 0),
                                         stop=(tap == 8),
                                         perf_mode=DR)
                for (n, half) in tiles:
                    evict(n, ob, half, pss[(n, half)])

            # ---- conv1 eviction: copy PSUM->y1 with sum, square with sumsq
            y1 = [[None] * CB for _ in range(IMGS)]

            def evict1(n, ob, half, ps):
                if y1[n][ob] is None:
                    y1[n][ob] = p_yz.tile([128, PIX], F32, tag="yz",
                                          name=f"y1_{n}_{ob}")
                idx = n * 2 + half
                ysl = y1[n][ob][:, half * HALF:(half + 1) * HALF]
                nc.scalar.activation(ysl, ps, AF.Copy, scale=2.0,
                                     accum_out=st1s[ob][:, idx:idx + 1])
                sq = p_sq.tile([128, HALF], F32, tag="sq")
                nc.vector.scalar_tensor_tensor(
                    out=sq, in0=ysl, scalar=1.0, in1=ysl,
                    op0=ALU.mult, op1=ALU.mult,
                    accum_out=st1q[ob][:, idx:idx + 1])

            # ---- stat reduce + AllReduce start for the given ob list.
            # cci rides the sync HWDGE queue: its completion semaphore fires
            # ~1us after issue (the SWDGE path takes ~8us), so the gpsimd
            # collective triggers almost immediately.
            def sync_stats(ss, qq, obs, nm):
                w = 2 * len(obs)
                pk = p_const.tile([128, w], F32, name=f"pk{nm}")
                for j, ob in enumerate(obs):
                    nc.vector.tensor_reduce(out=pk[:, 2 * j:2 * j + 1],
                                            in_=ss[ob],
                                            axis=mybir.AxisListType.X,
                                            op=ALU.add)
                    nc.vector.tensor_reduce(out=pk[:, 2 * j + 1:2 * j + 2],
                                            in_=qq[ob],
                                            axis=mybir.AxisListType.X,
                                            op=ALU.add)
                cci = p_dram.tile([128, w], F32, name=f"cci{nm}")
                cco = p_dram.tile([128, w], F32, name=f"cco{nm}")
                nc.sync.dma_start(out=cci, in_=pk)
                # poke: a tiny gpsimd op dependent on pk lands the gpsimd
                # sequencer right at stats-close, so the collective's
                # semaphore wait starts (and is observed) promptly
                poke = p_const.tile([128, 1], F32, name=f"poke{nm}")
                nc.gpsimd.tensor_scalar(out=poke, in0=pk[:, 0:1], scalar1=1.0,
                                        scalar2=None, op0=ALU.mult)
                nc.gpsimd.collective_compute(
                    "AllReduce", ALU.add, replica_groups=groups,
                    ins=[cci.opt()], outs=[cco.opt()])
                return cco

            def fetch_stats(cco, nm, w=2):
                red = p_const.tile([128, w], F32, name=f"red{nm}")
                nc.sync.dma_start(out=red, in_=cco)
                return red

            # ---- BN1 threshold from global sums (cols 2j:2j+2 of red) ----
            def thr_chain(red, j, ob, nm):
                m = p_const.tile([128, 1], F32, name=f"m{nm}")
                nc.vector.tensor_scalar(out=m, in0=red[:, 2 * j:2 * j + 1],
                                        scalar1=1.0 / NT, scalar2=None,
                                        op0=ALU.mult)
                mm = p_const.tile([128, 1], F32, name=f"mm{nm}")
                nc.vector.tensor_mul(mm, m, m)
                v = p_const.tile([128, 1], F32, name=f"v{nm}")
                nc.vector.scalar_tensor_tensor(
                    out=v, in0=red[:, 2 * j + 1:2 * j + 2], scalar=1.0 / NT,
                    in1=mm, op0=ALU.mult, op1=ALU.subtract)
                sd = p_const.tile([128, 1], F32, name=f"sd{nm}")
                nc.scalar.activation(sd, v, AF.Sqrt, bias=epsc)
                tb = p_const.tile([128, 1], F32, name=f"tb{nm}")
                nc.vector.tensor_mul(tb, bg1[:, ob:ob + 1], sd)
                thr = p_const.tile([128, 1], F32, name=f"thr{nm}")
                nc.vector.tensor_sub(thr, m, tb)
                return thr

            # binarize(BN1(y1[., ob])) == is_ge(y1, thr) - 0.5 into kb plane
            def binz_y(n, ob, thr):
                a4 = b2a[n].rearrange("p (k r c) -> p k r c", k=CB, r=HP)
                nc.vector.tensor_scalar(
                    out=a4[:, ob, 1:29, 1:29],
                    in0=y1[n][ob].rearrange("p (h w) -> p h w", h=H),
                    scalar1=thr, scalar2=0.5,
                    op0=ALU.is_ge, op1=ALU.subtract)

            # ================= conv1 (pair-major) =================
            wt1 = {}
            # signs of w1-ob0 go FIRST in the scalar queue, before any
            # throttled bulk-DMA issue instructions
            prep_weights(wst1_0, 1, 0, wt1)

            # ---- x: one DMA per (image, block), alternating the two HWDGE
            # queues; image n lands well before its pair's groups.
            # all x on the sync queue: its throttled issues block nothing,
            # and the scalar queue stays clear for the weight signs
            xt = [[None] * CB for _ in range(IMGS)]   # [n][b]
            for n in range(IMGS):
                for b in range(CB):
                    xr = p_x.tile([128, PIX], F32, tag="xp",
                                  name=f"x_{n}_{b}")
                    nc.sync.dma_start(
                        out=xr,
                        in_=x_in[n, b * 128:(b + 1) * 128].rearrange(
                            "c h w -> c (h w)"))
                    xt[n][b] = xr

            # w2 on sync behind the even x halves (needed only at conv1 end)
            wst2_0 = stage_w(w2_in, 0, "wst2_0", nc.sync)
            wst2_1 = stage_w(w2_in, 1, "wst2_1", nc.sync)

            # gamma/beta as [128, 2] (col = channel block), scalar queue
            def load_cvec(src, nm):
                t = p_const.tile([128, CB], F32, name=nm)
                nc.scalar.dma_start(out=t,
                                    in_=src.rearrange("(b p) -> p b", p=128))
                return t

            g1t = load_cvec(g1_in, "g1t")
            b1t = load_cvec(b1_in, "b1t")
            g2t = load_cvec(g2_in, "g2t")
            b2t = load_cvec(b2_in, "b2t")

            binz_x(0)
            binz_x(1)
            emit_group(wt1, xsign, evict1, 0, 0)
            prep_weights(wst1_1, 1, 1, wt1)
            for pair in range(4):
                if pair > 0:
                    emit_group(wt1, xsign, evict1, pair, 0)
                # next pair's binarize sits between this pair's ob0/ob1
                # eviction squares in the DVE queue: runs as soon as its x
                # slices land, never stalling the eviction pipeline
                if pair < 3:
                    binz_x(2 * pair + 2)
                    binz_x(2 * pair + 3)
                emit_group(wt1, xsign, evict1, pair, 1)

            # BN1: one AllReduce for both channel blocks
            cco1 = sync_stats(st1s, st1q, [0, 1], "1")
            # w2 prep fills the AllReduce wait on PE/Scalar
            wt2 = {}
            prep_weights(wst2_0, 2, 0, wt2)
            prep_weights(wst2_1, 2, 1, wt2)
            red1 = fetch_stats(cco1, "1", w=4)

            rg1 = p_const.tile([128, CB], F32, name="rg1")
            nc.vector.reciprocal(rg1, g1t)
            bg1 = p_const.tile([128, CB], F32, name="bg1")
            nc.vector.tensor_mul(bg1, b1t, rg1)
            thr1 = [thr_chain(red1, j, j, f"1_{j}") for j in range(CB)]

            # ---- conv2 eviction: z = 2*psum + x (fused sum), square.
            # ob0 squares on gpsimd (idle then) so the ob0 stats close
            # immediately after the last eviction; ob1 squares on scalar.
            z = [[None] * CB for _ in range(IMGS)]

            def evict2(n, ob, half, ps):
                if z[n][ob] is None:
                    z[n][ob] = p_yz.tile([128, PIX], F32, tag="yz",
                                         name=f"z_{n}_{ob}")
                idx = n * 2 + half
                zsl = z[n][ob][:, half * HALF:(half + 1) * HALF]
                nc.vector.scalar_tensor_tensor(
                    out=zsl, in0=ps, scalar=2.0,
                    in1=xt[n][ob][:, half * HALF:(half + 1) * HALF],
                    op0=ALU.mult, op1=ALU.add,
                    accum_out=st2s[ob][:, idx:idx + 1])
                sq = p_sq.tile([128, HALF], F32, tag="sq")
                nc.scalar.activation(sq, zsl, AF.Square,
                                     accum_out=st2q[ob][:, idx:idx + 1])

            # ---- final: out = clip(z * fscale + fbias), one DMA per (n,ob)
            def affine_chain(red, ob, nm):
                m = p_const.tile([128, 1], F32, name=f"m{nm}")
                nc.vector.tensor_scalar(out=m, in0=red[:, 0:1],
                                        scalar1=1.0 / NT, scalar2=None,
                                        op0=ALU.mult)
                mm = p_const.tile([128, 1], F32, name=f"mm{nm}")
                nc.vector.tensor_mul(mm, m, m)
                v = p_const.tile([128, 1], F32, name=f"v{nm}")
                nc.vector.scalar_tensor_tensor(
                    out=v, in0=red[:, 1:2], scalar=1.0 / NT, in1=mm,
                    op0=ALU.mult, op1=ALU.subtract)
                sd = p_const.tile([128, 1], F32, name=f"sd{nm}")
                nc.scalar.activation(sd, v, AF.Sqrt, bias=epsc)
                rstd = p_const.tile([128, 1], F32, name=f"rstd{nm}")
                nc.vector.reciprocal(rstd, sd)
                fsc = p_const.tile([128, 1], F32, name=f"fsc{nm}")
                nc.vector.tensor_mul(fsc, g2t[:, ob:ob + 1], rstd)
                msc = p_const.tile([128, 1], F32, name=f"msc{nm}")
                nc.vector.tensor_mul(msc, m, fsc)
                fb = p_const.tile([128, 1], F32, name=f"fb{nm}")
                nc.vector.tensor_sub(fb, b2t[:, ob:ob + 1], msc)
                return fsc, fb

            def store_out(n, ob, fsc, fb, eng):
                o1 = p_o1.tile([128, PIX], F32, tag="o1")
                nc.scalar.activation(o1, z[n][ob], AF.Identity,
                                     bias=fb, scale=fsc)
                nc.vector.tensor_scalar(out=o1, in0=o1, scalar1=-1.0,
                                        scalar2=1.0, op0=ALU.max,
                                        op1=ALU.min)
                eng.dma_start(
                    out=out_d[n, ob * 128:(ob + 1) * 128].rearrange(
                        "c h w -> c (h w)"),
                    in_=o1)

            # ================= conv2 (ob-major) =================
            # binarize paced with the first ob0 groups
            for pair in range(4):
                for n in (2 * pair, 2 * pair + 1):
                    binz_y(n, 0, thr1[0])
                    binz_y(n, 1, thr1[1])
                emit_group(wt2, b2a, evict2, pair, 0)

            cco2a = sync_stats(st2s, st2q, [0], "2a")

            emit_group(wt2, b2a, evict2, 0, 1)
            emit_group(wt2, b2a, evict2, 1, 1)
            emit_group(wt2, b2a, evict2, 2, 1)
            # AllReduce(2a) done by now: ob0 affine+clamp+store under conv2-ob1
            red2a = fetch_stats(cco2a, "2a")
            fsc0, fb0 = affine_chain(red2a, 0, "2a")
            for n in range(IMGS):
                store_out(n, 0, fsc0, fb0, nc.sync if n % 2 == 0 else nc.gpsimd)
            emit_group(wt2, b2a, evict2, 3, 1)

            cco2b = sync_stats(st2s, st2q, [1], "2b")
            red2b = fetch_stats(cco2b, "2b")
            fsc1, fb1 = affine_chain(red2b, 1, "2b")
            for n in range(IMGS):
                store_out(n, 1, fsc1, fb1, nc.sync if n % 2 == 0 else nc.gpsimd)

    nc.compile()
    return nc


def _get_program():
    global _PROGRAM
    if _PROGRAM is None:
        _PROGRAM = _build_program()
    return _PROGRAM


def run_sharded(inputs, **spmd_kwargs):
    """Shard inputs across 8 cores, run, and gather. Returns (out, results)."""
    nc = _get_program()
    x = np.ascontiguousarray(np.asarray(inputs["x"], dtype=np.float32))
    base = {
        k: np.ascontiguousarray(np.asarray(inputs[k], dtype=np.float32))
        for k in ("w1", "w2", "gamma1", "beta1", "gamma2", "beta2")
    }
    shards = np.split(x, N_CORES, axis=0)
    in_maps = [{"x": shards[i], **base} for i in range(N_CORES)]
    res = run_bass_kernel_spmd(nc, in_maps, core_ids=list(range(N_CORES)),
                               **spmd_kwargs)
    out = np.concatenate([res.results[i]["out"] for i in range(N_CORES)],
                         axis=0).astype(np.float32)
    return out, res


def kernel(**inputs):
    out, _ = run_sharded(inputs)
    return out
